# revision 10
# baseline (speedup 1.0000x reference)
"""Trainium2 Bass kernel for nn_Luong_61684320305412 (bidirectional masked
softmax attention, B=8, L0=L1=2048, D=256).

Sharding: data-parallel over batch B across the 8 NeuronCores. Per core:

    S    = q0 @ q1^T + (-448 m0) outer (448 m1)     [fp8e4 operands]
    E    = exp(S/256)                                [fp16, masked entries -> 0]
    out0 = (E  @ [qn1 | 1]) -> cols 0:256 / col 256  (qn = q/16, fp16)
    out1 = (E^T @ [qn0 | 1]) -> cols 0:256 / col 256

Implementation notes:
  - S matmuls run in fp8e4 with MatmulPerfMode.DoubleRow: K=256 in one
    instruction ([128p, 2, .] operand layout), 2 rhs elems/cycle.
  - The mask outer product is a separate K=1 fp8 matmul per 512-chunk
    (lhsT=-128*m0, rhs=+64*m1 -> exp arg -32 -> exp==0 after the fp16 cast;
    more-negative args make the scalar engine's exp produce NaN).
  - E is needed in both orientations (lhsT for both out matmuls); S is
    computed twice (S and S^T) from the same fp8 operands with roles
    swapped - numerically identical to transposing E.
  - exp runs on the scalar engine psum->SBUF in [128,1024] chunks,
    writing E in fp16 (out matmuls are fp16 x fp16 -> f32 psum).
  - Softmax denominators come from two ones-columns appended to the
    fp16 rhs (psum cols 256/257), divided out on the vector engine.
  - Out accumulation chains are emitted pairwise-interleaved across two
    psum banks to hide PSUM read-modify-write latency.
"""

import math
from contextlib import ExitStack

import numpy as np

import concourse.bass as bass
import concourse.tile as tile
from concourse import bacc, mybir
from concourse.bass_utils import run_bass_kernel_spmd

P = 128
B = 8
L = 2048          # L0 == L1
D = 256
T = L // P        # 16 row tiles
AUGW = D + 2      # 258: fp16 q/16 augmented with two ones columns
MASK0C = -128.0   # (-128 m0)*(64 m1)/256 = -32: exp(-32)=1.3e-14 stays a
MASK1C = 64.0     # normal f32 on the Act engine (extreme args NaN there) and
                  # flushes to exactly 0.0 in the fp16 cast of E
SCALE2 = 1.0 / 256.0   # applied to scores inside exp
INV16 = 1.0 / 16.0

f32 = mybir.dt.float32
f32r = mybir.dt.float32r
f16 = mybir.dt.float16
f8 = mybir.dt.float8e4
i32 = mybir.dt.int32
MUL = mybir.AluOpType.mult
EXP = mybir.ActivationFunctionType.Exp
DR = mybir.MatmulPerfMode.DoubleRow


def _emit(tc: tile.TileContext, ctx: ExitStack, io: dict):
    nc = tc.nc
    q0, q1, m0, m1 = io["q0"], io["q1"], io["mask0"], io["mask1"]
    out0, out1 = io["out0"], io["out1"]

    consts = ctx.enter_context(tc.tile_pool(name="consts", bufs=1))
    stage = ctx.enter_context(tc.tile_pool(name="stage", bufs=3))
    qpool = ctx.enter_context(tc.tile_pool(name="qpool", bufs=1))
    e_pool = ctx.enter_context(tc.tile_pool(name="e", bufs=1))
    outp = ctx.enter_context(tc.tile_pool(name="outp", bufs=4))
    small = ctx.enter_context(tc.tile_pool(name="small", bufs=4))
    s_psum = ctx.enter_context(tc.tile_pool(name="s_psum", bufs=2, space="PSUM"))
    t_psum = ctx.enter_context(tc.tile_pool(name="t_psum", bufs=2, space="PSUM"))
    o_psum = ctx.enter_context(tc.tile_pool(name="o_psum", bufs=2, space="PSUM"))

    # ---- persistent operand tiles ----
    # fp16 q/16 with ones cols at 256,257 (rhs of the out matmuls)
    q0a = qpool.tile([P, T, AUGW], f16)
    q1a = qpool.tile([P, T, AUGW], f16)
    # fp8 raw-q in DoubleRow layout [d%128, d//128, l] (S matmul operands)
    q0t = qpool.tile([P, 2, L], f8)
    q1t = qpool.tile([P, 2, L], f8)
    # E in both orientations, fp16
    e0 = e_pool.tile([P, T, L], f16)  # [l0, l1]
    e1 = e_pool.tile([P, T, L], f16)  # [l1, l0]

    nc.vector.memset(q0a[:, :, D:AUGW], 1.0)
    nc.vector.memset(q1a[:, :, D:AUGW], 1.0)

    # ---- masks: int32 [L] -> fp8 rows scaled -+448 ----
    m0i = consts.tile([1, L], i32)
    m1i = consts.tile([1, L], i32)
    nc.sync.dma_start(out=m0i, in_=m0.rearrange("(o l) -> o l", o=1))
    nc.sync.dma_start(out=m1i, in_=m1.rearrange("(o l) -> o l", o=1))
    m0f = consts.tile([1, L], f32)
    m1f = consts.tile([1, L], f32)
    nc.vector.tensor_copy(out=m0f, in_=m0i)
    nc.vector.tensor_copy(out=m1f, in_=m1i)
    xm0 = consts.tile([1, L], f8)
    xm1 = consts.tile([1, L], f8)
    nc.vector.tensor_scalar_mul(out=xm0, in0=m0f, scalar1=MASK0C)
    nc.vector.tensor_scalar_mul(out=xm1, in0=m1f, scalar1=MASK1C)

    # ---- load q stripes, build fp16 aug (q/16) + fp8 transposed layouts ----
    from concourse.masks import make_identity
    ident_f = consts.tile([P, P], f32)
    make_identity(nc, ident_f)
    ident16 = consts.tile([P, P], f16)
    nc.vector.tensor_copy(out=ident16, in_=ident_f)

    for t in range(T):
        for src, aug, tr in ((q0, q0a, q0t), (q1, q1a, q1t)):
            st = stage.tile([P, D], f32, tag="st")
            nc.sync.dma_start(
                out=st, in_=src.rearrange("(t p) d -> t p d", p=P)[t]
            )
            # fp16 q/16 (gpsimd to keep DVE free)
            nc.gpsimd.tensor_scalar_mul(out=aug[:, t, 0:D], in0=st, scalar1=INV16)
            # fp8 raw q, transposed to [d, l] via PE (transpose runs in fp16,
            # DVE casts the psum tile down to fp8 on copy-out)
            st16 = stage.tile([P, D], f16, tag="st16")
            nc.vector.tensor_copy(out=st16, in_=st)
            for dc in range(2):
                pt = t_psum.tile([P, P], f16, tag="tp")
                nc.tensor.transpose(pt, st16[:, dc * P:(dc + 1) * P], ident16)
                nc.vector.tensor_copy(
                    out=tr[:, dc, t * P:(t + 1) * P], in_=pt
                )

    # ---- S matmuls + exp, one orientation at a time ----
    def s_phase(lT, rT, lm, rm, edst):
        # E stripes [128, L] for one orientation; psum in [128,1024] halves
        for t in range(T):
            for h in range(2):
                ps = s_psum.tile([P, 1024], f32, tag="sp")
                for c in range(2):
                    off = h * 1024 + c * 512
                    nc.tensor.matmul(
                        ps[:, c * 512:(c + 1) * 512],
                        lhsT=lT[:, :, t * P:(t + 1) * P],
                        rhs=rT[:, :, off:off + 512],
                        start=True, stop=False, perf_mode=DR,
                    )
                for c in range(2):
                    off = h * 1024 + c * 512
                    nc.tensor.matmul(
                        ps[:, c * 512:(c + 1) * 512],
                        lhsT=lm[:, t * P:(t + 1) * P],
                        rhs=rm[:, off:off + 512],
                        start=False, stop=True,
                    )
                nc.scalar.activation(
                    out=edst[:, t, h * 1024:(h + 1) * 1024], in_=ps,
                    func=EXP, scale=SCALE2,
                )

    # ---- out matmuls: pairwise-interleaved accumulation chains ----
    def out_phase(esrc, raug, odram):
        for j0 in range(0, T, 2):
            pos = [o_psum.tile([P, AUGW], f32, tag="op", name=f"op{_k}") for _k in range(2)]
            for t in range(T):
                for k in range(2):
                    j = j0 + k
                    nc.tensor.matmul(
                        pos[k],
                        lhsT=esrc[:, t, j * P:(j + 1) * P],
                        rhs=raug[:, t, :],
                        start=(t == 0), stop=(t == T - 1),
                    )
            for k in range(2):
                j = j0 + k
                rc = small.tile([P, 1], f32, tag="rc")
                nc.vector.reciprocal(rc, pos[k][:, D:D + 1])
                ot = outp.tile([P, D], f32, tag="ot")
                nc.vector.tensor_scalar_mul(out=ot, in0=pos[k][:, 0:D], scalar1=rc)
                nc.sync.dma_start(out=odram[j * P:(j + 1) * P, :], in_=ot)

    if "dbg_e0" in io:
        pass  # taps added below
    s_phase(q0t, q1t, xm0, xm1, e0)
    if "dbg_e0" in io:
        with tc.tile_pool(name="dbgp", bufs=1) as dbgp:
            etap = dbgp.tile([P, 2048], f32)
            nc.vector.tensor_copy(out=etap, in_=e0[:, 0, :])
            nc.sync.dma_start(out=io["dbg_e0"], in_=etap)
            qtap = dbgp.tile([P, AUGW], f32)
            nc.vector.tensor_copy(out=qtap, in_=q0a[:, 0, :])
            nc.sync.dma_start(out=io["dbg_q0a"], in_=qtap)
    # interleave S^T with out1 on the PE stream
    for t in range(T):
        # S^T stripe t
        for h in range(2):
            ps = s_psum.tile([P, 1024], f32, tag="sp")
            for c in range(2):
                off = h * 1024 + c * 512
                nc.tensor.matmul(
                    ps[:, c * 512:(c + 1) * 512],
                    lhsT=q1t[:, :, t * P:(t + 1) * P],
                    rhs=q0t[:, :, off:off + 512],
                    start=True, stop=False, perf_mode=DR,
                )
            for c in range(2):
                off = h * 1024 + c * 512
                nc.tensor.matmul(
                    ps[:, c * 512:(c + 1) * 512],
                    lhsT=xm1[:, t * P:(t + 1) * P],
                    rhs=xm0[:, off:off + 512],
                    start=False, stop=True,
                )
            nc.scalar.activation(
                out=e1[:, t, h * 1024:(h + 1) * 1024], in_=ps,
                func=EXP, scale=SCALE2,
            )
        # out1 tile t (chain over all E0 stripes; E0 fully exp'd by now-ish)
        if t % 2 == 1:
            j0 = t - 1
            pos = [o_psum.tile([P, AUGW], f32, tag="op", name=f"op{_k}") for _k in range(2)]
            for tt in range(T):
                for k in range(2):
                    j = j0 + k
                    nc.tensor.matmul(
                        pos[k],
                        lhsT=e0[:, tt, j * P:(j + 1) * P],
                        rhs=q0a[:, tt, :],
                        start=(tt == 0), stop=(tt == T - 1),
                    )
            for k in range(2):
                j = j0 + k
                rc = small.tile([P, 1], f32, tag="rc")
                nc.vector.reciprocal(rc, pos[k][:, D:D + 1])
                ot = outp.tile([P, D], f32, tag="ot")
                nc.vector.tensor_scalar_mul(out=ot, in0=pos[k][:, 0:D], scalar1=rc)
                nc.sync.dma_start(out=out1[j * P:(j + 1) * P, :], in_=ot)
    out_phase(e1, q1a, out0)


_CACHED_NC = None


def _build(debug=False):
    global _CACHED_NC
    if _CACHED_NC is not None:
        return _CACHED_NC
    nc = bacc.Bacc("TRN2", target_bir_lowering=False, debug=False)
    io = {
        "q0": nc.dram_tensor("q0", [L, D], f32, kind="ExternalInput").ap(),
        "q1": nc.dram_tensor("q1", [L, D], f32, kind="ExternalInput").ap(),
        "mask0": nc.dram_tensor("mask0", [L], i32, kind="ExternalInput").ap(),
        "mask1": nc.dram_tensor("mask1", [L], i32, kind="ExternalInput").ap(),
        "out0": nc.dram_tensor("out0", [L, D], f32, kind="ExternalOutput").ap(),
        "out1": nc.dram_tensor("out1", [L, D], f32, kind="ExternalOutput").ap(),
    }
    if debug:
        io["dbg_e0"] = nc.dram_tensor("dbg_e0", [P, 2048], f32, kind="ExternalOutput").ap()
        io["dbg_q0a"] = nc.dram_tensor("dbg_q0a", [P, AUGW], f32, kind="ExternalOutput").ap()
    with tile.TileContext(nc) as tc:
        with ExitStack() as ctx:
            _emit(tc, ctx, io)
    nc.compile()
    _CACHED_NC = nc
    return nc


def run_on_cores(q0, q1, mask0, mask1, trace=False):
    """Run the SPMD kernel; returns (out0, out1, BassKernelResults)."""
    nc = _build()
    in_maps = [
        {
            "q0": np.ascontiguousarray(q0[b], dtype=np.float32),
            "q1": np.ascontiguousarray(q1[b], dtype=np.float32),
            "mask0": np.ascontiguousarray(mask0[b], dtype=np.int32),
            "mask1": np.ascontiguousarray(mask1[b], dtype=np.int32),
        }
        for b in range(B)
    ]
    br = run_bass_kernel_spmd(nc, in_maps, list(range(B)), trace=trace)
    out0 = np.stack([br.results[b]["out0"] for b in range(B)])
    out1 = np.stack([br.results[b]["out1"] for b in range(B)])
    return out0, out1, br


def kernel(q0, q1, len0=None, len1=None, mask0=None, mask1=None, **_):
    q0 = np.asarray(q0, dtype=np.float32)
    q1 = np.asarray(q1, dtype=np.float32)
    mask0 = np.asarray(mask0, dtype=np.int32)
    mask1 = np.asarray(mask1, dtype=np.int32)
    out0, out1, _br = run_on_cores(q0, q1, mask0, mask1, trace=False)
    return out0, out1


# revision 12
# speedup vs baseline: 1.3856x; 1.3856x over previous
"""Trainium2 Bass kernel for nn_Luong_61684320305412 (bidirectional masked
softmax attention, B=8, L0=L1=2048, D=256).

Sharding: data-parallel over batch B across the 8 NeuronCores. Per core:

    S    = q0 @ q1^T + (-448 m0) outer (448 m1)     [fp8e4 operands]
    E    = exp(S/256)                                [fp16, masked entries -> 0]
    out0 = (E  @ [qn1 | 1]) -> cols 0:256 / col 256  (qn = q/16, fp16)
    out1 = (E^T @ [qn0 | 1]) -> cols 0:256 / col 256

Implementation notes:
  - S matmuls run in fp8e4 with MatmulPerfMode.DoubleRow: K=256 in one
    instruction ([128p, 2, .] operand layout), 2 rhs elems/cycle.
  - The mask outer product is a separate K=1 fp8 matmul per 512-chunk
    (lhsT=-128*m0, rhs=+64*m1 -> exp arg -32 -> exp==0 after the fp16 cast;
    more-negative args make the scalar engine's exp produce NaN).
  - E is needed in both orientations (lhsT for both out matmuls); S is
    computed twice (S and S^T) from the same fp8 operands with roles
    swapped - numerically identical to transposing E.
  - exp runs on the scalar engine psum->SBUF in [128,1024] chunks,
    writing E in fp16 (out matmuls are fp16 x fp16 -> f32 psum).
  - Softmax denominators come from two ones-columns appended to the
    fp16 rhs (psum cols 256/257), divided out on the vector engine.
  - Out accumulation chains are emitted pairwise-interleaved across two
    psum banks to hide PSUM read-modify-write latency.
"""

import math
from contextlib import ExitStack

import numpy as np

import concourse.bass as bass
import concourse.tile as tile
from concourse import bacc, mybir
from concourse.bass_utils import run_bass_kernel_spmd

P = 128
B = 8
L = 2048          # L0 == L1
D = 256
T = L // P        # 16 row tiles
AUGW = D + 2      # 258: fp16 q/16 augmented with two ones columns
MASK0C = -128.0   # (-128 m0)*(64 m1)/256 = -32: exp(-32)=1.3e-14 stays a
MASK1C = 64.0     # normal f32 on the Act engine (extreme args NaN there) and
                  # flushes to exactly 0.0 in the fp16 cast of E
SCALE2 = 1.0 / 256.0   # applied to scores inside exp
INV16 = 1.0 / 16.0

f32 = mybir.dt.float32
f32r = mybir.dt.float32r
f16 = mybir.dt.float16
f8 = mybir.dt.float8e4
i32 = mybir.dt.int32
MUL = mybir.AluOpType.mult
EXP = mybir.ActivationFunctionType.Exp
DR = mybir.MatmulPerfMode.DoubleRow


def _emit(tc: tile.TileContext, ctx: ExitStack, io: dict):
    nc = tc.nc
    q0, q1, m0, m1 = io["q0"], io["q1"], io["mask0"], io["mask1"]
    out0, out1 = io["out0"], io["out1"]

    consts = ctx.enter_context(tc.tile_pool(name="consts", bufs=1))
    stage = ctx.enter_context(tc.tile_pool(name="stage", bufs=3))
    qpool = ctx.enter_context(tc.tile_pool(name="qpool", bufs=1))
    e_pool = ctx.enter_context(tc.tile_pool(name="e", bufs=1))
    outp = ctx.enter_context(tc.tile_pool(name="outp", bufs=4))
    small = ctx.enter_context(tc.tile_pool(name="small", bufs=4))
    s_psum = ctx.enter_context(tc.tile_pool(name="s_psum", bufs=2, space="PSUM"))
    t_psum = ctx.enter_context(tc.tile_pool(name="t_psum", bufs=1, space="PSUM"))
    o_psum = ctx.enter_context(tc.tile_pool(name="o_psum", bufs=3, space="PSUM"))

    # ---- persistent operand tiles ----
    # fp16 q/16 with ones cols at 256,257 (rhs of the out matmuls)
    q0a = qpool.tile([P, T, AUGW], f16)
    q1a = qpool.tile([P, T, AUGW], f16)
    # fp8 raw-q in DoubleRow layout [d%128, d//128, l] (S matmul operands)
    q0t = qpool.tile([P, 2, L], f8)
    q1t = qpool.tile([P, 2, L], f8)
    # E in both orientations, fp16
    e0 = e_pool.tile([P, T, L], f16)  # [l0, l1]
    e1 = e_pool.tile([P, T, L], f16)  # [l1, l0]

    nc.vector.memset(q0a[:, :, D:AUGW], 1.0)
    nc.vector.memset(q1a[:, :, D:AUGW], 1.0)

    # ---- masks: int32 [L] -> fp8 rows ----
    m0i = consts.tile([1, L], i32)
    m1i = consts.tile([1, L], i32)
    nc.sync.dma_start(out=m0i, in_=m0.rearrange("(o l) -> o l", o=1))
    nc.sync.dma_start(out=m1i, in_=m1.rearrange("(o l) -> o l", o=1))
    m0f = consts.tile([1, L], f32)
    m1f = consts.tile([1, L], f32)
    nc.vector.tensor_copy(out=m0f, in_=m0i)
    nc.vector.tensor_copy(out=m1f, in_=m1i)
    xm0 = consts.tile([1, L], f8)
    xm1 = consts.tile([1, L], f8)
    nc.vector.tensor_scalar_mul(out=xm0, in0=m0f, scalar1=MASK0C)
    nc.vector.tensor_scalar_mul(out=xm1, in0=m1f, scalar1=MASK1C)

    from concourse.masks import make_identity
    ident_f = consts.tile([P, P], f32)
    make_identity(nc, ident_f)
    ident16 = consts.tile([P, P], f16)
    nc.vector.tensor_copy(out=ident16, in_=ident_f)

    # ---- per-stripe load + cast + PE transpose into the fp8 DR layout ----
    def prep_stripe(src, aug, tr, t):
        st = stage.tile([P, D], f32, tag="st")
        nc.sync.dma_start(out=st, in_=src.rearrange("(t p) d -> t p d", p=P)[t])
        nc.vector.tensor_scalar_mul(out=aug[:, t, 0:D], in0=st, scalar1=INV16)
        st16 = stage.tile([P, D], f16, tag="st16")
        nc.vector.tensor_copy(out=st16, in_=st)
        for dc in range(2):
            pt = t_psum.tile([P, P], f16, tag="tp")
            nc.tensor.transpose(pt, st16[:, dc * P:(dc + 1) * P], ident16)
            nc.vector.tensor_copy(out=tr[:, dc, t * P:(t + 1) * P], in_=pt)

    # ---- S matmuls + exp for one stripe of one orientation ----
    def s_stripe(lT, rT, lm, rm, edst, t):
        for h in range(2):
            ps = s_psum.tile([P, 1024], f32, tag="sp")
            for c in range(2):
                off = h * 1024 + c * 512
                nc.tensor.matmul(
                    ps[:, c * 512:(c + 1) * 512],
                    lhsT=lT[:, :, t * P:(t + 1) * P],
                    rhs=rT[:, :, off:off + 512],
                    start=True, stop=False, perf_mode=DR,
                )
            for c in range(2):
                off = h * 1024 + c * 512
                nc.tensor.matmul(
                    ps[:, c * 512:(c + 1) * 512],
                    lhsT=lm[:, t * P:(t + 1) * P],
                    rhs=rm[:, off:off + 512],
                    start=False, stop=True,
                )
            nc.scalar.activation(
                out=edst[:, t, h * 1024:(h + 1) * 1024], in_=ps,
                func=EXP, scale=SCALE2,
            )

    # ---- one pairwise-interleaved pair of out accumulation chains ----
    def out_pair(esrc, raug, odram, j0):
        pos = [o_psum.tile([P, AUGW], f32, tag="op", name=f"op{_k}") for _k in range(2)]
        for t in range(T):
            for k in range(2):
                j = j0 + k
                nc.tensor.matmul(
                    pos[k],
                    lhsT=esrc[:, t, j * P:(j + 1) * P],
                    rhs=raug[:, t, :],
                    start=(t == 0), stop=(t == T - 1),
                )
        for k in range(2):
            j = j0 + k
            rc = small.tile([P, 1], f32, tag="rc")
            nc.vector.reciprocal(rc, pos[k][:, D:D + 1])
            ot = outp.tile([P, D], f32, tag="ot")
            nc.vector.tensor_scalar_mul(out=ot, in0=pos[k][:, 0:D], scalar1=rc)
            nc.sync.dma_start(out=odram[j * P:(j + 1) * P, :], in_=ot)

    # ---- emission schedule ----
    # q1 first (S stripes sweep all of q1t as rhs), then q0 stripe t feeds S(t)
    for t in range(T):
        prep_stripe(q1, q1a, q1t, t)
    for t in range(T):
        prep_stripe(q0, q0a, q0t, t)
        s_stripe(q0t, q1t, xm0, xm1, e0, t)
    # S^T stripes interleaved with out1 pairs (out1 consumes e0)
    for t in range(T):
        s_stripe(q1t, q0t, xm1, xm0, e1, t)
        if t % 2 == 1:
            out_pair(e0, q0a, out1, t - 1)
    for j0 in range(0, T, 2):
        out_pair(e1, q1a, out0, j0)



_CACHED_NC = None


def _build(debug=False):
    global _CACHED_NC
    if _CACHED_NC is not None:
        return _CACHED_NC
    nc = bacc.Bacc("TRN2", target_bir_lowering=False, debug=False)
    io = {
        "q0": nc.dram_tensor("q0", [L, D], f32, kind="ExternalInput").ap(),
        "q1": nc.dram_tensor("q1", [L, D], f32, kind="ExternalInput").ap(),
        "mask0": nc.dram_tensor("mask0", [L], i32, kind="ExternalInput").ap(),
        "mask1": nc.dram_tensor("mask1", [L], i32, kind="ExternalInput").ap(),
        "out0": nc.dram_tensor("out0", [L, D], f32, kind="ExternalOutput").ap(),
        "out1": nc.dram_tensor("out1", [L, D], f32, kind="ExternalOutput").ap(),
    }
    if debug:
        io["dbg_e0"] = nc.dram_tensor("dbg_e0", [P, 2048], f32, kind="ExternalOutput").ap()
        io["dbg_q0a"] = nc.dram_tensor("dbg_q0a", [P, AUGW], f32, kind="ExternalOutput").ap()
    with tile.TileContext(nc) as tc:
        with ExitStack() as ctx:
            _emit(tc, ctx, io)
    nc.compile()
    _CACHED_NC = nc
    return nc


def run_on_cores(q0, q1, mask0, mask1, trace=False):
    """Run the SPMD kernel; returns (out0, out1, BassKernelResults)."""
    nc = _build()
    in_maps = [
        {
            "q0": np.ascontiguousarray(q0[b], dtype=np.float32),
            "q1": np.ascontiguousarray(q1[b], dtype=np.float32),
            "mask0": np.ascontiguousarray(mask0[b], dtype=np.int32),
            "mask1": np.ascontiguousarray(mask1[b], dtype=np.int32),
        }
        for b in range(B)
    ]
    br = run_bass_kernel_spmd(nc, in_maps, list(range(B)), trace=trace)
    out0 = np.stack([br.results[b]["out0"] for b in range(B)])
    out1 = np.stack([br.results[b]["out1"] for b in range(B)])
    return out0, out1, br


def kernel(q0, q1, len0=None, len1=None, mask0=None, mask1=None, **_):
    q0 = np.asarray(q0, dtype=np.float32)
    q1 = np.asarray(q1, dtype=np.float32)
    mask0 = np.asarray(mask0, dtype=np.int32)
    mask1 = np.asarray(mask1, dtype=np.int32)
    out0, out1, _br = run_on_cores(q0, q1, mask0, mask1, trace=False)
    return out0, out1


# revision 13
# speedup vs baseline: 1.4164x; 1.0222x over previous
"""Trainium2 Bass kernel for nn_Luong_61684320305412 (bidirectional masked
softmax attention, B=8, L0=L1=2048, D=256).

Sharding: data-parallel over batch B across the 8 NeuronCores. Per core:

    S    = q0 @ q1^T + (-448 m0) outer (448 m1)     [fp8e4 operands]
    E    = exp(S/256)                                [fp16, masked entries -> 0]
    out0 = (E  @ [qn1 | 1]) -> cols 0:256 / col 256  (qn = q/16, fp16)
    out1 = (E^T @ [qn0 | 1]) -> cols 0:256 / col 256

Implementation notes:
  - S matmuls run in fp8e4 with MatmulPerfMode.DoubleRow: K=256 in one
    instruction ([128p, 2, .] operand layout), 2 rhs elems/cycle.
  - The mask outer product is a separate K=1 fp8 matmul per 512-chunk
    (lhsT=-128*m0, rhs=+64*m1 -> exp arg -32 -> exp==0 after the fp16 cast;
    more-negative args make the scalar engine's exp produce NaN).
  - E is needed in both orientations (lhsT for both out matmuls); S is
    computed twice (S and S^T) from the same fp8 operands with roles
    swapped - numerically identical to transposing E.
  - exp runs on the scalar engine psum->SBUF in [128,1024] chunks,
    writing E in fp16 (out matmuls are fp16 x fp16 -> f32 psum).
  - Softmax denominators come from two ones-columns appended to the
    fp16 rhs (psum cols 256/257), divided out on the vector engine.
  - Out accumulation chains are emitted pairwise-interleaved across two
    psum banks to hide PSUM read-modify-write latency.
"""

import math
from contextlib import ExitStack

import numpy as np

import concourse.bass as bass
import concourse.tile as tile
from concourse import bacc, mybir
from concourse.bass_utils import run_bass_kernel_spmd

P = 128
B = 8
L = 2048          # L0 == L1
D = 256
T = L // P        # 16 row tiles
AUGW = D + 2      # 258: fp16 q/16 augmented with two ones columns
MASK0C = -128.0   # (-128 m0)*(64 m1)/256 = -32: exp(-32)=1.3e-14 stays a
MASK1C = 64.0     # normal f32 on the Act engine (extreme args NaN there) and
                  # flushes to exactly 0.0 in the fp16 cast of E
SCALE2 = 1.0 / 256.0   # applied to scores inside exp
INV16 = 1.0 / 16.0

f32 = mybir.dt.float32
f32r = mybir.dt.float32r
f16 = mybir.dt.float16
f8 = mybir.dt.float8e4
i32 = mybir.dt.int32
MUL = mybir.AluOpType.mult
EXP = mybir.ActivationFunctionType.Exp
DR = mybir.MatmulPerfMode.DoubleRow


def _emit(tc: tile.TileContext, ctx: ExitStack, io: dict):
    nc = tc.nc
    q0, q1, m0, m1 = io["q0"], io["q1"], io["mask0"], io["mask1"]
    out0, out1 = io["out0"], io["out1"]

    consts = ctx.enter_context(tc.tile_pool(name="consts", bufs=1))
    stage = ctx.enter_context(tc.tile_pool(name="stage", bufs=3))
    qpool = ctx.enter_context(tc.tile_pool(name="qpool", bufs=1))
    e_pool = ctx.enter_context(tc.tile_pool(name="e", bufs=1))
    outp = ctx.enter_context(tc.tile_pool(name="outp", bufs=4))
    small = ctx.enter_context(tc.tile_pool(name="small", bufs=4))
    s_psum = ctx.enter_context(tc.tile_pool(name="s_psum", bufs=2, space="PSUM"))
    t_psum = ctx.enter_context(tc.tile_pool(name="t_psum", bufs=1, space="PSUM"))
    o_psum = ctx.enter_context(tc.tile_pool(name="o_psum", bufs=3, space="PSUM"))

    # ---- persistent operand tiles ----
    # fp16 q/16 with ones cols at 256,257 (rhs of the out matmuls)
    q0a = qpool.tile([P, T, AUGW], f16)
    q1a = qpool.tile([P, T, AUGW], f16)
    # fp8 raw-q in DoubleRow layout [d%128, d//128, l] (S matmul operands)
    q0t = qpool.tile([P, 2, L], f8)
    q1t = qpool.tile([P, 2, L], f8)
    # E in both orientations, fp16
    e0 = e_pool.tile([P, T, L], f16)  # [l0, l1]
    e1 = e_pool.tile([P, T, L], f16)  # [l1, l0]

    nc.vector.memset(q0a[:, :, D:AUGW], 1.0)
    nc.vector.memset(q1a[:, :, D:AUGW], 1.0)

    # ---- masks: int32 [L] -> fp8 rows ----
    m0i = consts.tile([1, L], i32)
    m1i = consts.tile([1, L], i32)
    nc.sync.dma_start(out=m0i, in_=m0.rearrange("(o l) -> o l", o=1))
    nc.sync.dma_start(out=m1i, in_=m1.rearrange("(o l) -> o l", o=1))
    m0f = consts.tile([1, L], f32)
    m1f = consts.tile([1, L], f32)
    nc.vector.tensor_copy(out=m0f, in_=m0i)
    nc.vector.tensor_copy(out=m1f, in_=m1i)
    xm0 = consts.tile([1, L], f8)
    xm1 = consts.tile([1, L], f8)
    nc.vector.tensor_scalar_mul(out=xm0, in0=m0f, scalar1=MASK0C)
    nc.vector.tensor_scalar_mul(out=xm1, in0=m1f, scalar1=MASK1C)

    from concourse.masks import make_identity
    ident_f = consts.tile([P, P], f32)
    make_identity(nc, ident_f)
    ident16 = consts.tile([P, P], f16)
    nc.vector.tensor_copy(out=ident16, in_=ident_f)

    # ---- per-stripe load + cast + PE transpose into the fp8 DR layout ----
    # on_act: run the f32->fp16 casts on the scalar engine (it is idle during
    # the prep burst) instead of DVE, which also handles the psum copy-outs.
    def prep_stripe(src, aug, tr, t, on_act):
        st = stage.tile([P, D], f32, tag="st")
        nc.sync.dma_start(out=st, in_=src.rearrange("(t p) d -> t p d", p=P)[t])
        st16 = stage.tile([P, D], f16, tag="st16")
        if on_act:
            nc.scalar.mul(aug[:, t, 0:D], st, INV16)
            nc.scalar.copy(st16, st)
        else:
            nc.vector.tensor_scalar_mul(out=aug[:, t, 0:D], in0=st, scalar1=INV16)
            nc.vector.tensor_copy(out=st16, in_=st)
        for dc in range(2):
            pt = t_psum.tile([P, P], f16, tag="tp")
            nc.tensor.transpose(pt, st16[:, dc * P:(dc + 1) * P], ident16)
            nc.vector.tensor_copy(out=tr[:, dc, t * P:(t + 1) * P], in_=pt)

    # ---- S matmuls + exp for one stripe of one orientation ----
    def s_stripe(lT, rT, lm, rm, edst, t):
        # batch the 4 DR matmuls before the 4 mask matmuls: each accumulate
        # partner sits 3 instructions behind its RAW dependency, so the
        # weight loads prefetch instead of serializing.
        pss = [s_psum.tile([P, 1024], f32, tag="sp", name=f"sp{_h}") for _h in range(2)]
        for h in range(2):
            for c in range(2):
                off = h * 1024 + c * 512
                nc.tensor.matmul(
                    pss[h][:, c * 512:(c + 1) * 512],
                    lhsT=lT[:, :, t * P:(t + 1) * P],
                    rhs=rT[:, :, off:off + 512],
                    start=True, stop=False, perf_mode=DR,
                )
        for h in range(2):
            for c in range(2):
                off = h * 1024 + c * 512
                nc.tensor.matmul(
                    pss[h][:, c * 512:(c + 1) * 512],
                    lhsT=lm[:, t * P:(t + 1) * P],
                    rhs=rm[:, off:off + 512],
                    start=False, stop=True,
                )
        for h in range(2):
            nc.scalar.activation(
                out=edst[:, t, h * 1024:(h + 1) * 1024], in_=pss[h],
                func=EXP, scale=SCALE2,
            )

    # ---- one pairwise-interleaved pair of out accumulation chains ----
    def out_pair(esrc, raug, odram, j0):
        pos = [o_psum.tile([P, AUGW], f32, tag="op", name=f"op{_k}") for _k in range(2)]
        for t in range(T):
            for k in range(2):
                j = j0 + k
                nc.tensor.matmul(
                    pos[k],
                    lhsT=esrc[:, t, j * P:(j + 1) * P],
                    rhs=raug[:, t, :],
                    start=(t == 0), stop=(t == T - 1),
                )
        for k in range(2):
            j = j0 + k
            rc = small.tile([P, 1], f32, tag="rc")
            nc.vector.reciprocal(rc, pos[k][:, D:D + 1])
            ot = outp.tile([P, D], f32, tag="ot")
            nc.vector.tensor_scalar_mul(out=ot, in0=pos[k][:, 0:D], scalar1=rc)
            nc.sync.dma_start(out=odram[j * P:(j + 1) * P, :], in_=ot)

    # ---- emission schedule ----
    # q1 first (S stripes sweep all of q1t as rhs), then q0 stripe t feeds S(t)
    for t in range(T):
        prep_stripe(q1, q1a, q1t, t, on_act=True)
    for t in range(T):
        prep_stripe(q0, q0a, q0t, t, on_act=False)
        s_stripe(q0t, q1t, xm0, xm1, e0, t)
    # S^T stripes interleaved with out1 pairs (out1 consumes e0)
    for t in range(T):
        s_stripe(q1t, q0t, xm1, xm0, e1, t)
        if t % 2 == 1:
            out_pair(e0, q0a, out1, t - 1)
    for j0 in range(0, T, 2):
        out_pair(e1, q1a, out0, j0)



_CACHED_NC = None


def _build(debug=False):
    global _CACHED_NC
    if _CACHED_NC is not None:
        return _CACHED_NC
    nc = bacc.Bacc("TRN2", target_bir_lowering=False, debug=False)
    io = {
        "q0": nc.dram_tensor("q0", [L, D], f32, kind="ExternalInput").ap(),
        "q1": nc.dram_tensor("q1", [L, D], f32, kind="ExternalInput").ap(),
        "mask0": nc.dram_tensor("mask0", [L], i32, kind="ExternalInput").ap(),
        "mask1": nc.dram_tensor("mask1", [L], i32, kind="ExternalInput").ap(),
        "out0": nc.dram_tensor("out0", [L, D], f32, kind="ExternalOutput").ap(),
        "out1": nc.dram_tensor("out1", [L, D], f32, kind="ExternalOutput").ap(),
    }
    if debug:
        io["dbg_e0"] = nc.dram_tensor("dbg_e0", [P, 2048], f32, kind="ExternalOutput").ap()
        io["dbg_q0a"] = nc.dram_tensor("dbg_q0a", [P, AUGW], f32, kind="ExternalOutput").ap()
    with tile.TileContext(nc) as tc:
        with ExitStack() as ctx:
            _emit(tc, ctx, io)
    nc.compile()
    _CACHED_NC = nc
    return nc


def run_on_cores(q0, q1, mask0, mask1, trace=False):
    """Run the SPMD kernel; returns (out0, out1, BassKernelResults)."""
    nc = _build()
    in_maps = [
        {
            "q0": np.ascontiguousarray(q0[b], dtype=np.float32),
            "q1": np.ascontiguousarray(q1[b], dtype=np.float32),
            "mask0": np.ascontiguousarray(mask0[b], dtype=np.int32),
            "mask1": np.ascontiguousarray(mask1[b], dtype=np.int32),
        }
        for b in range(B)
    ]
    br = run_bass_kernel_spmd(nc, in_maps, list(range(B)), trace=trace)
    out0 = np.stack([br.results[b]["out0"] for b in range(B)])
    out1 = np.stack([br.results[b]["out1"] for b in range(B)])
    return out0, out1, br


def kernel(q0, q1, len0=None, len1=None, mask0=None, mask1=None, **_):
    q0 = np.asarray(q0, dtype=np.float32)
    q1 = np.asarray(q1, dtype=np.float32)
    mask0 = np.asarray(mask0, dtype=np.int32)
    mask1 = np.asarray(mask1, dtype=np.int32)
    out0, out1, _br = run_on_cores(q0, q1, mask0, mask1, trace=False)
    return out0, out1


# revision 14
# speedup vs baseline: 1.5601x; 1.1015x over previous
"""Trainium2 Bass kernel for nn_Luong_61684320305412 (bidirectional masked
softmax attention, B=8, L0=L1=2048, D=256).

Sharding: data-parallel over batch B across the 8 NeuronCores. Per core:

    S    = q0 @ q1^T + (-448 m0) outer (448 m1)     [fp8e4 operands]
    E    = exp(S/256)                                [fp16, masked entries -> 0]
    out0 = (E  @ [qn1 | 1]) -> cols 0:256 / col 256  (qn = q/16, fp16)
    out1 = (E^T @ [qn0 | 1]) -> cols 0:256 / col 256

Implementation notes:
  - S matmuls run in fp8e4 with MatmulPerfMode.DoubleRow: K=256 in one
    instruction ([128p, 2, .] operand layout), 2 rhs elems/cycle.
  - The mask outer product is a separate K=1 fp8 matmul per 512-chunk
    (lhsT=-128*m0, rhs=+64*m1 -> exp arg -32 -> exp==0 after the fp16 cast;
    more-negative args make the scalar engine's exp produce NaN).
  - E is needed in both orientations (lhsT for both out matmuls); S is
    computed twice (S and S^T) from the same fp8 operands with roles
    swapped - numerically identical to transposing E.
  - exp runs on the scalar engine psum->SBUF in [128,1024] chunks,
    writing E in fp16 (out matmuls are fp16 x fp16 -> f32 psum).
  - Softmax denominators come from two ones-columns appended to the
    fp16 rhs (psum cols 256/257), divided out on the vector engine.
  - Out accumulation chains are emitted pairwise-interleaved across two
    psum banks to hide PSUM read-modify-write latency.
"""

import math
from contextlib import ExitStack

import numpy as np

import concourse.bass as bass
import concourse.tile as tile
from concourse import bacc, mybir
from concourse.bass_utils import run_bass_kernel_spmd

P = 128
B = 8
L = 2048          # L0 == L1
D = 256
T = L // P        # 16 row tiles
AUGW = D + 2      # 258: fp16 q/16 augmented with two ones columns
MASK0C = -128.0   # (-128 m0)*(64 m1)/256 = -32: exp(-32)=1.3e-14 stays a
MASK1C = 64.0     # normal f32 on the Act engine (extreme args NaN there) and
                  # flushes to exactly 0.0 in the fp16 cast of E
SCALE2 = 1.0 / 256.0   # applied to scores inside exp
INV16 = 1.0 / 16.0

f32 = mybir.dt.float32
f32r = mybir.dt.float32r
f16 = mybir.dt.float16
f8 = mybir.dt.float8e4
i32 = mybir.dt.int32
MUL = mybir.AluOpType.mult
EXP = mybir.ActivationFunctionType.Exp
DR = mybir.MatmulPerfMode.DoubleRow


def _emit(tc: tile.TileContext, ctx: ExitStack, io: dict):
    nc = tc.nc
    q0, q1, m0, m1 = io["q0"], io["q1"], io["mask0"], io["mask1"]
    out0, out1 = io["out0"], io["out1"]

    consts = ctx.enter_context(tc.tile_pool(name="consts", bufs=1))
    stage = ctx.enter_context(tc.tile_pool(name="stage", bufs=4))
    stage16 = ctx.enter_context(tc.tile_pool(name="stage16", bufs=6))
    qpool = ctx.enter_context(tc.tile_pool(name="qpool", bufs=1))
    e_pool = ctx.enter_context(tc.tile_pool(name="e", bufs=1))
    outp = ctx.enter_context(tc.tile_pool(name="outp", bufs=4))
    small = ctx.enter_context(tc.tile_pool(name="small", bufs=4))
    s_psum = ctx.enter_context(tc.tile_pool(name="s_psum", bufs=2, space="PSUM"))
    t_psum = ctx.enter_context(tc.tile_pool(name="t_psum", bufs=2, space="PSUM"))
    o_psum = ctx.enter_context(tc.tile_pool(name="o_psum", bufs=2, space="PSUM"))

    # ---- persistent operand tiles ----
    q0a = qpool.tile([P, T, AUGW], f16)   # q/16 | ones cols (out-matmul rhs)
    q1a = qpool.tile([P, T, AUGW], f16)
    q0t = qpool.tile([P, 2, L], f8)       # raw q, [d%128, d//128, l] DR layout
    q1t = qpool.tile([P, 2, L], f8)
    e0 = e_pool.tile([P, T, L], f16)      # E  [l0, l1]
    e1 = e_pool.tile([P, T, L], f16)      # E^T [l1, l0] (built by PE transpose)

    nc.vector.memset(q0a[:, :, D:AUGW], 1.0)
    nc.vector.memset(q1a[:, :, D:AUGW], 1.0)

    # ---- masks: int32 [L] -> fp8 rows ----
    m0i = consts.tile([1, L], i32)
    m1i = consts.tile([1, L], i32)
    nc.sync.dma_start(out=m0i, in_=m0.rearrange("(o l) -> o l", o=1))
    nc.sync.dma_start(out=m1i, in_=m1.rearrange("(o l) -> o l", o=1))
    m0f = consts.tile([1, L], f32)
    m1f = consts.tile([1, L], f32)
    nc.vector.tensor_copy(out=m0f, in_=m0i)
    nc.vector.tensor_copy(out=m1f, in_=m1i)
    xm0 = consts.tile([1, L], f8)
    xm1 = consts.tile([1, L], f8)
    nc.vector.tensor_scalar_mul(out=xm0, in0=m0f, scalar1=MASK0C)
    nc.vector.tensor_scalar_mul(out=xm1, in0=m1f, scalar1=MASK1C)

    from concourse.masks import make_identity
    ident_f = consts.tile([P, P], f32)
    make_identity(nc, ident_f)
    ident16 = consts.tile([P, P], f16)
    nc.vector.tensor_copy(out=ident16, in_=ident_f)

    # ---- load q, cast, and transpose into the fp8 DR layout ----
    # 4 stripes at a time: 8 PE transposes pack one [128,1024] fp16 psum tile,
    # drained by a single wide DVE copy. q1's f32->fp16 casts run on the
    # scalar engine (idle during prep); q0's on DVE.
    def prep_pack(src, aug, tr, p4, on_act):
        pt = t_psum.tile([P, 1024], f16, tag="tp")
        for ti in range(4):
            t = p4 * 4 + ti
            st = stage.tile([P, D], f32, tag="st")
            nc.sync.dma_start(
                out=st, in_=src.rearrange("(t p) d -> t p d", p=P)[t]
            )
            st16 = stage16.tile([P, D], f16, tag="st16")
            if on_act:
                nc.scalar.mul(aug[:, t, 0:D], st, INV16)
                nc.scalar.copy(st16, st)
            else:
                nc.vector.tensor_scalar_mul(out=aug[:, t, 0:D], in0=st, scalar1=INV16)
                nc.vector.tensor_copy(out=st16, in_=st)
            for dc in range(2):
                nc.tensor.transpose(
                    pt[:, (ti * 2 + dc) * P:(ti * 2 + dc + 1) * P],
                    st16[:, dc * P:(dc + 1) * P], ident16,
                )
        dst = tr[:, :, p4 * 512:(p4 + 1) * 512]
        dstv = dst.rearrange("p two (t f) -> p t two f", t=4)
        srcv = pt.rearrange("p (t two f) -> p t two f", t=4, two=2)
        nc.vector.tensor_copy(out=dstv, in_=srcv)

    # ---- S matmuls + exp for one stripe (orientation 0 only) ----
    def s_stripe(t):
        pss = [s_psum.tile([P, 1024], f32, tag="sp", name=f"sp{_h}") for _h in range(2)]
        for h in range(2):
            for c in range(2):
                off = h * 1024 + c * 512
                nc.tensor.matmul(
                    pss[h][:, c * 512:(c + 1) * 512],
                    lhsT=q0t[:, :, t * P:(t + 1) * P],
                    rhs=q1t[:, :, off:off + 512],
                    start=True, stop=False, perf_mode=DR,
                )
        for h in range(2):
            for c in range(2):
                off = h * 1024 + c * 512
                nc.tensor.matmul(
                    pss[h][:, c * 512:(c + 1) * 512],
                    lhsT=xm0[:, t * P:(t + 1) * P],
                    rhs=xm1[:, off:off + 512],
                    start=False, stop=True,
                )
        for h in range(2):
            nc.scalar.activation(
                out=e0[:, t, h * 1024:(h + 1) * 1024], in_=pss[h],
                func=EXP, scale=SCALE2,
            )

    # ---- build E^T stripe s by transposing 16 E tiles on the PE ----
    def et_stripe(s):
        for half in range(2):
            pt = t_psum.tile([P, 1024], f16, tag="tp")
            for i in range(8):
                t = half * 8 + i
                nc.tensor.transpose(
                    pt[:, i * P:(i + 1) * P],
                    e0[:, t, s * P:(s + 1) * P], ident16,
                )
            nc.vector.tensor_copy(
                out=e1[:, s, half * 1024:(half + 1) * 1024], in_=pt
            )

    # ---- one pairwise-interleaved pair of out accumulation chains ----
    def out_pair(esrc, raug, odram, j0):
        pos = [o_psum.tile([P, AUGW], f32, tag="op", name=f"op{_k}") for _k in range(2)]
        for t in range(T):
            for k in range(2):
                j = j0 + k
                nc.tensor.matmul(
                    pos[k],
                    lhsT=esrc[:, t, j * P:(j + 1) * P],
                    rhs=raug[:, t, :],
                    start=(t == 0), stop=(t == T - 1),
                )
        for k in range(2):
            j = j0 + k
            rc = small.tile([P, 1], f32, tag="rc")
            nc.vector.reciprocal(rc, pos[k][:, D:D + 1])
            ot = outp.tile([P, D], f32, tag="ot")
            nc.vector.tensor_scalar_mul(out=ot, in0=pos[k][:, 0:D], scalar1=rc)
            nc.sync.dma_start(out=odram[j * P:(j + 1) * P, :], in_=ot)

    # ---- emission schedule ----
    for p4 in range(4):
        prep_pack(q1, q1a, q1t, p4, on_act=True)
    for p4 in range(4):
        prep_pack(q0, q0a, q0t, p4, on_act=False)
    for t in range(T):
        s_stripe(t)
    for s in range(T):
        et_stripe(s)
    for j0 in range(0, T, 2):
        out_pair(e0, q0a, out1, j0)
        out_pair(e1, q1a, out0, j0)

_CACHED_NC = None


def _build(debug=False):
    global _CACHED_NC
    if _CACHED_NC is not None:
        return _CACHED_NC
    nc = bacc.Bacc("TRN2", target_bir_lowering=False, debug=False)
    io = {
        "q0": nc.dram_tensor("q0", [L, D], f32, kind="ExternalInput").ap(),
        "q1": nc.dram_tensor("q1", [L, D], f32, kind="ExternalInput").ap(),
        "mask0": nc.dram_tensor("mask0", [L], i32, kind="ExternalInput").ap(),
        "mask1": nc.dram_tensor("mask1", [L], i32, kind="ExternalInput").ap(),
        "out0": nc.dram_tensor("out0", [L, D], f32, kind="ExternalOutput").ap(),
        "out1": nc.dram_tensor("out1", [L, D], f32, kind="ExternalOutput").ap(),
    }
    if debug:
        io["dbg_e0"] = nc.dram_tensor("dbg_e0", [P, 2048], f32, kind="ExternalOutput").ap()
        io["dbg_q0a"] = nc.dram_tensor("dbg_q0a", [P, AUGW], f32, kind="ExternalOutput").ap()
    with tile.TileContext(nc) as tc:
        with ExitStack() as ctx:
            _emit(tc, ctx, io)
    nc.compile()
    _CACHED_NC = nc
    return nc


def run_on_cores(q0, q1, mask0, mask1, trace=False):
    """Run the SPMD kernel; returns (out0, out1, BassKernelResults)."""
    nc = _build()
    in_maps = [
        {
            "q0": np.ascontiguousarray(q0[b], dtype=np.float32),
            "q1": np.ascontiguousarray(q1[b], dtype=np.float32),
            "mask0": np.ascontiguousarray(mask0[b], dtype=np.int32),
            "mask1": np.ascontiguousarray(mask1[b], dtype=np.int32),
        }
        for b in range(B)
    ]
    br = run_bass_kernel_spmd(nc, in_maps, list(range(B)), trace=trace)
    out0 = np.stack([br.results[b]["out0"] for b in range(B)])
    out1 = np.stack([br.results[b]["out1"] for b in range(B)])
    return out0, out1, br


def kernel(q0, q1, len0=None, len1=None, mask0=None, mask1=None, **_):
    q0 = np.asarray(q0, dtype=np.float32)
    q1 = np.asarray(q1, dtype=np.float32)
    mask0 = np.asarray(mask0, dtype=np.int32)
    mask1 = np.asarray(mask1, dtype=np.int32)
    out0, out1, _br = run_on_cores(q0, q1, mask0, mask1, trace=False)
    return out0, out1


# revision 15
# speedup vs baseline: 1.7281x; 1.1077x over previous
"""Trainium2 Bass kernel for nn_Luong_61684320305412 (bidirectional masked
softmax attention, B=8, L0=L1=2048, D=256).

Sharding: data-parallel over batch B across the 8 NeuronCores. Per core:

    S    = q0 @ q1^T + (-448 m0) outer (448 m1)     [fp8e4 operands]
    E    = exp(S/256)                                [fp16, masked entries -> 0]
    out0 = (E  @ [qn1 | 1]) -> cols 0:256 / col 256  (qn = q/16, fp16)
    out1 = (E^T @ [qn0 | 1]) -> cols 0:256 / col 256

Implementation notes:
  - S matmuls run in fp8e4 with MatmulPerfMode.DoubleRow: K=256 in one
    instruction ([128p, 2, .] operand layout), 2 rhs elems/cycle.
  - The mask outer product is a separate K=1 fp8 matmul per 512-chunk
    (lhsT=-128*m0, rhs=+64*m1 -> exp arg -32 -> exp==0 after the fp16 cast;
    more-negative args make the scalar engine's exp produce NaN).
  - E is needed in both orientations (lhsT for both out matmuls); S is
    computed twice (S and S^T) from the same fp8 operands with roles
    swapped - numerically identical to transposing E.
  - exp runs on the scalar engine psum->SBUF in [128,1024] chunks,
    writing E in fp16 (out matmuls are fp16 x fp16 -> f32 psum).
  - Softmax denominators come from two ones-columns appended to the
    fp16 rhs (psum cols 256/257), divided out on the vector engine.
  - Out accumulation chains are emitted pairwise-interleaved across two
    psum banks to hide PSUM read-modify-write latency.
"""

import math
from contextlib import ExitStack

import numpy as np

import concourse.bass as bass
import concourse.tile as tile
from concourse import bacc, mybir
from concourse.bass_utils import run_bass_kernel_spmd

P = 128
B = 8
L = 2048          # L0 == L1
D = 256
T = L // P        # 16 row tiles
AUGW = D + 2      # 258: fp16 q/16 augmented with two ones columns
HUGE = 60000.0    # fp16-exact; mask select: min(HUGE*(1-m1[j]) + HUGE*(1-m0[p]), E)
                  # = 0 iff m0[p]=m1[j]=1 else E (E >= 0 always)
SCALE2 = 1.0 / 256.0   # applied to scores inside exp
INV16 = 1.0 / 16.0

f32 = mybir.dt.float32
f32r = mybir.dt.float32r
f16 = mybir.dt.float16
f8 = mybir.dt.float8e4
i32 = mybir.dt.int32
MUL = mybir.AluOpType.mult
EXP = mybir.ActivationFunctionType.Exp
DR = mybir.MatmulPerfMode.DoubleRow


def _emit(tc: tile.TileContext, ctx: ExitStack, io: dict):
    nc = tc.nc
    q0, q1, m0, m1 = io["q0"], io["q1"], io["mask0"], io["mask1"]
    out0, out1 = io["out0"], io["out1"]

    consts = ctx.enter_context(tc.tile_pool(name="consts", bufs=1))
    stage = ctx.enter_context(tc.tile_pool(name="stage", bufs=4))
    stage16 = ctx.enter_context(tc.tile_pool(name="stage16", bufs=6))
    qpool = ctx.enter_context(tc.tile_pool(name="qpool", bufs=1))
    e_pool = ctx.enter_context(tc.tile_pool(name="e", bufs=1))
    outp = ctx.enter_context(tc.tile_pool(name="outp", bufs=4))
    small = ctx.enter_context(tc.tile_pool(name="small", bufs=4))
    s_psum = ctx.enter_context(tc.tile_pool(name="s_psum", bufs=2, space="PSUM"))
    t_psum = ctx.enter_context(tc.tile_pool(name="t_psum", bufs=2, space="PSUM"))
    o_psum = ctx.enter_context(tc.tile_pool(name="o_psum", bufs=2, space="PSUM"))

    # ---- persistent operand tiles ----
    q0a = qpool.tile([P, T, AUGW], f16)   # q/16 | ones cols (out-matmul rhs)
    q1a = qpool.tile([P, T, AUGW], f16)
    q0t = qpool.tile([P, 2, L], f8)       # raw q, [d%128, d//128, l] DR layout
    q1t = qpool.tile([P, 2, L], f8)
    e0 = e_pool.tile([P, T, L], f16)      # E  [l0, l1]
    e1 = e_pool.tile([P, T, L], f16)      # E^T [l1, l0] (built by PE transpose)

    nc.vector.memset(q0a[:, :, D:AUGW], 1.0)
    nc.vector.memset(q1a[:, :, D:AUGW], 1.0)

    # ---- mask prep ----
    # row tile: wm1[0, j] = HUGE*(1-m1[j]), broadcast to all partitions by a
    # one-time K=1 PE outer product; col tile: s0[p, t] = HUGE*(1-m0[t*128+p])
    m1i = consts.tile([1, L], i32)
    nc.sync.dma_start(out=m1i, in_=m1.rearrange("(o l) -> o l", o=1))
    m1f = consts.tile([1, L], f32)
    nc.vector.tensor_copy(out=m1f, in_=m1i)
    wm1row = consts.tile([1, L], f16)
    nc.vector.tensor_scalar(out=wm1row, in0=m1f, scalar1=-HUGE, scalar2=HUGE,
                            op0=MUL, op1=mybir.AluOpType.add)
    onesrow = consts.tile([1, P], f16)
    nc.vector.memset(onesrow, 1.0)
    wm1 = consts.tile([P, L], f16)

    m0i = consts.tile([P, T], i32)
    nc.sync.dma_start(out=m0i, in_=m0.rearrange("(t p) -> p t", p=P))
    m0fc = consts.tile([P, T], f32)
    nc.vector.tensor_copy(out=m0fc, in_=m0i)
    s0 = consts.tile([P, T], f32)
    nc.vector.tensor_scalar(out=s0, in0=m0fc, scalar1=-HUGE, scalar2=HUGE,
                            op0=MUL, op1=mybir.AluOpType.add)

    from concourse.masks import make_identity
    ident_f = consts.tile([P, P], f32)
    make_identity(nc, ident_f)
    ident16 = consts.tile([P, P], f16)
    nc.vector.tensor_copy(out=ident16, in_=ident_f)

    # broadcast wm1row -> wm1 via PE outer product (one-time)
    for c in range(4):
        pw = s_psum.tile([P, 512], f32, tag="sp", name=f"pw{c}")
        nc.tensor.matmul(pw, lhsT=onesrow, rhs=wm1row[:, c * 512:(c + 1) * 512],
                         start=True, stop=True)
        nc.scalar.copy(wm1[:, c * 512:(c + 1) * 512], pw)

    # ---- load q, cast, and transpose into the fp8 DR layout ----
    def prep_pack(src, aug, tr, p4, on_act):
        pt = t_psum.tile([P, 1024], f16, tag="tp")
        for ti in range(4):
            t = p4 * 4 + ti
            st = stage.tile([P, D], f32, tag="st")
            nc.sync.dma_start(
                out=st, in_=src.rearrange("(t p) d -> t p d", p=P)[t]
            )
            st16 = stage16.tile([P, D], f16, tag="st16")
            if on_act:
                nc.scalar.mul(aug[:, t, 0:D], st, INV16)
                nc.scalar.copy(st16, st)
            else:
                nc.vector.tensor_scalar_mul(out=aug[:, t, 0:D], in0=st, scalar1=INV16)
                nc.vector.tensor_copy(out=st16, in_=st)
            for dc in range(2):
                nc.tensor.transpose(
                    pt[:, (ti * 2 + dc) * P:(ti * 2 + dc + 1) * P],
                    st16[:, dc * P:(dc + 1) * P], ident16,
                )
        dst = tr[:, :, p4 * 512:(p4 + 1) * 512]
        dstv = dst.rearrange("p two (t f) -> p t two f", t=4)
        srcv = pt.rearrange("p (t two f) -> p t two f", t=4, two=2)
        nc.vector.tensor_copy(out=dstv, in_=srcv)

    # ---- S matmuls (pure DR, no accumulation) + exp + fused mask-select ----
    def s_stripe(t):
        pss = [s_psum.tile([P, 1024], f32, tag="sp", name=f"sp{_h}") for _h in range(2)]
        for h in range(2):
            for c in range(2):
                off = h * 1024 + c * 512
                nc.tensor.matmul(
                    pss[h][:, c * 512:(c + 1) * 512],
                    lhsT=q0t[:, :, t * P:(t + 1) * P],
                    rhs=q1t[:, :, off:off + 512],
                    start=True, stop=True, perf_mode=DR,
                )
        for h in range(2):
            nc.scalar.activation(
                out=e0[:, t, h * 1024:(h + 1) * 1024], in_=pss[h],
                func=EXP, scale=SCALE2,
            )
        # masked entries -> exactly 0: e0 = min(wm1 + s0[p], e0)
        nc.vector.scalar_tensor_tensor(
            out=e0[:, t, :], in0=wm1, scalar=s0[:, t:t + 1], in1=e0[:, t, :],
            op0=mybir.AluOpType.add, op1=mybir.AluOpType.min,
        )

    # ---- E^T tiles for source stripe i (consumable stripe-by-stripe) ----
    def et_source(i):
        for half in range(2):
            pt = t_psum.tile([P, 1024], f16, tag="tp")
            for si in range(8):
                s = half * 8 + si
                nc.tensor.transpose(
                    pt[:, si * P:(si + 1) * P],
                    e0[:, i, s * P:(s + 1) * P], ident16,
                )
            dst = e1[:, half * 8:(half + 1) * 8, i * P:(i + 1) * P]
            nc.vector.tensor_copy(
                out=dst, in_=pt.rearrange("p (s f) -> p s f", s=8)
            )

    # ---- one pairwise-interleaved pair of out accumulation chains ----
    def out_pair(esrc, raug, odram, j0):
        pos = [o_psum.tile([P, AUGW], f32, tag="op", name=f"op{_k}") for _k in range(2)]
        for t in range(T):
            for k in range(2):
                j = j0 + k
                nc.tensor.matmul(
                    pos[k],
                    lhsT=esrc[:, t, j * P:(j + 1) * P],
                    rhs=raug[:, t, :],
                    start=(t == 0), stop=(t == T - 1),
                )
        for k in range(2):
            j = j0 + k
            rc = small.tile([P, 1], f32, tag="rc")
            nc.vector.reciprocal(rc, pos[k][:, D:D + 1])
            ot = outp.tile([P, D], f32, tag="ot")
            nc.scalar.mul(ot, pos[k][:, 0:D], rc)
            nc.sync.dma_start(out=odram[j * P:(j + 1) * P, :], in_=ot)

    # ---- emission schedule ----
    for p4 in range(4):
        prep_pack(q1, q1a, q1t, p4, on_act=True)
    for p4 in range(4):
        prep_pack(q0, q0a, q0t, p4, on_act=False)
    for t in range(T):
        s_stripe(t)
        et_source(t)
    for j0 in range(0, T, 2):
        out_pair(e0, q0a, out1, j0)
        out_pair(e1, q1a, out0, j0)

_CACHED_NC = None


def _build(debug=False):
    global _CACHED_NC
    if _CACHED_NC is not None:
        return _CACHED_NC
    nc = bacc.Bacc("TRN2", target_bir_lowering=False, debug=False)
    io = {
        "q0": nc.dram_tensor("q0", [L, D], f32, kind="ExternalInput").ap(),
        "q1": nc.dram_tensor("q1", [L, D], f32, kind="ExternalInput").ap(),
        "mask0": nc.dram_tensor("mask0", [L], i32, kind="ExternalInput").ap(),
        "mask1": nc.dram_tensor("mask1", [L], i32, kind="ExternalInput").ap(),
        "out0": nc.dram_tensor("out0", [L, D], f32, kind="ExternalOutput").ap(),
        "out1": nc.dram_tensor("out1", [L, D], f32, kind="ExternalOutput").ap(),
    }
    if debug:
        io["dbg_e0"] = nc.dram_tensor("dbg_e0", [P, 2048], f32, kind="ExternalOutput").ap()
        io["dbg_q0a"] = nc.dram_tensor("dbg_q0a", [P, AUGW], f32, kind="ExternalOutput").ap()
    with tile.TileContext(nc) as tc:
        with ExitStack() as ctx:
            _emit(tc, ctx, io)
    nc.compile()
    _CACHED_NC = nc
    return nc


def run_on_cores(q0, q1, mask0, mask1, trace=False):
    """Run the SPMD kernel; returns (out0, out1, BassKernelResults)."""
    nc = _build()
    in_maps = [
        {
            "q0": np.ascontiguousarray(q0[b], dtype=np.float32),
            "q1": np.ascontiguousarray(q1[b], dtype=np.float32),
            "mask0": np.ascontiguousarray(mask0[b], dtype=np.int32),
            "mask1": np.ascontiguousarray(mask1[b], dtype=np.int32),
        }
        for b in range(B)
    ]
    br = run_bass_kernel_spmd(nc, in_maps, list(range(B)), trace=trace)
    out0 = np.stack([br.results[b]["out0"] for b in range(B)])
    out1 = np.stack([br.results[b]["out1"] for b in range(B)])
    return out0, out1, br


def kernel(q0, q1, len0=None, len1=None, mask0=None, mask1=None, **_):
    q0 = np.asarray(q0, dtype=np.float32)
    q1 = np.asarray(q1, dtype=np.float32)
    mask0 = np.asarray(mask0, dtype=np.int32)
    mask1 = np.asarray(mask1, dtype=np.int32)
    out0, out1, _br = run_on_cores(q0, q1, mask0, mask1, trace=False)
    return out0, out1


# revision 17
# speedup vs baseline: 1.8766x; 1.0859x over previous
"""Trainium2 Bass kernel for nn_Luong_61684320305412 (bidirectional masked
softmax attention, B=8, L0=L1=2048, D=256).

Sharding: data-parallel over batch B across the 8 NeuronCores. Per core:

    S    = q0 @ q1^T + (-448 m0) outer (448 m1)     [fp8e4 operands]
    E    = exp(S/256)                                [fp16, masked entries -> 0]
    out0 = (E  @ [qn1 | 1]) -> cols 0:256 / col 256  (qn = q/16, fp16)
    out1 = (E^T @ [qn0 | 1]) -> cols 0:256 / col 256

Implementation notes:
  - S matmuls run in fp8e4 with MatmulPerfMode.DoubleRow: K=256 in one
    instruction ([128p, 2, .] operand layout), 2 rhs elems/cycle.
  - The mask outer product is a separate K=1 fp8 matmul per 512-chunk
    (lhsT=-128*m0, rhs=+64*m1 -> exp arg -32 -> exp==0 after the fp16 cast;
    more-negative args make the scalar engine's exp produce NaN).
  - E is needed in both orientations (lhsT for both out matmuls); S is
    computed twice (S and S^T) from the same fp8 operands with roles
    swapped - numerically identical to transposing E.
  - exp runs on the scalar engine psum->SBUF in [128,1024] chunks,
    writing E in fp16 (out matmuls are fp16 x fp16 -> f32 psum).
  - Softmax denominators come from two ones-columns appended to the
    fp16 rhs (psum cols 256/257), divided out on the vector engine.
  - Out accumulation chains are emitted pairwise-interleaved across two
    psum banks to hide PSUM read-modify-write latency.
"""

import math
from contextlib import ExitStack

import numpy as np

import concourse.bass as bass
import concourse.tile as tile
from concourse import bacc, mybir
from concourse.bass_utils import run_bass_kernel_spmd

P = 128
B = 8
L = 2048          # L0 == L1
D = 256
T = L // P        # 16 row tiles
AUGW = D + 2      # 258: fp16 q/16 augmented with two ones columns
HUGE = 60000.0    # fp16-exact; mask select: min(HUGE*(1-m1[j]) + HUGE*(1-m0[p]), E)
                  # = 0 iff m0[p]=m1[j]=1 else E (E >= 0 always)
SCALE2 = 1.0 / 256.0   # applied to scores inside exp
INV16 = 1.0 / 16.0

f32 = mybir.dt.float32
f32r = mybir.dt.float32r
f16 = mybir.dt.float16
f8 = mybir.dt.float8e4
i32 = mybir.dt.int32
MUL = mybir.AluOpType.mult
EXP = mybir.ActivationFunctionType.Exp
DR = mybir.MatmulPerfMode.DoubleRow


def _emit(tc: tile.TileContext, ctx: ExitStack, io: dict):
    nc = tc.nc
    q0, q1, m0, m1 = io["q0"], io["q1"], io["mask0"], io["mask1"]
    out0, out1 = io["out0"], io["out1"]

    consts = ctx.enter_context(tc.tile_pool(name="consts", bufs=1))
    stage = ctx.enter_context(tc.tile_pool(name="stage", bufs=4))
    stage16 = ctx.enter_context(tc.tile_pool(name="stage16", bufs=6))
    qpool = ctx.enter_context(tc.tile_pool(name="qpool", bufs=1))
    e_pool = ctx.enter_context(tc.tile_pool(name="e", bufs=1))
    outp = ctx.enter_context(tc.tile_pool(name="outp", bufs=4))
    small = ctx.enter_context(tc.tile_pool(name="small", bufs=4))
    s_psum = ctx.enter_context(tc.tile_pool(name="s_psum", bufs=2, space="PSUM"))
    t_psum = ctx.enter_context(tc.tile_pool(name="t_psum", bufs=2, space="PSUM"))
    o_psum = ctx.enter_context(tc.tile_pool(name="o_psum", bufs=2, space="PSUM"))

    # ---- persistent operand tiles ----
    q0a = qpool.tile([P, T, AUGW], f16)   # q/16 | ones cols (out-matmul rhs)
    q1a = qpool.tile([P, T, AUGW], f16)
    q0t = qpool.tile([P, 2, L], f8)       # raw q, [d%128, d//128, l] DR layout
    q1t = qpool.tile([P, 2, L], f8)
    e0 = e_pool.tile([P, T, L], f16)      # E  [l0, l1]
    e1 = e_pool.tile([P, T, L], f16)      # E^T [l1, l0] (built by PE transpose)

    nc.vector.memset(q0a[:, :, D:AUGW], 1.0)
    nc.vector.memset(q1a[:, :, D:AUGW], 1.0)

    # ---- mask prep ----
    # row tile: wm1[0, j] = HUGE*(1-m1[j]), broadcast to all partitions by a
    # one-time K=1 PE outer product; col tile: s0[p, t] = HUGE*(1-m0[t*128+p])
    m1i = consts.tile([1, L], i32)
    nc.sync.dma_start(out=m1i, in_=m1.rearrange("(o l) -> o l", o=1))
    m1f = consts.tile([1, L], f32)
    nc.vector.tensor_copy(out=m1f, in_=m1i)
    wm1row = consts.tile([1, L], f16)
    nc.vector.tensor_scalar(out=wm1row, in0=m1f, scalar1=-HUGE, scalar2=HUGE,
                            op0=MUL, op1=mybir.AluOpType.add)
    onesrow = consts.tile([1, P], f16)
    nc.vector.memset(onesrow, 1.0)
    wm1 = consts.tile([P, L], f16)

    m0i = consts.tile([P, T], i32)
    nc.sync.dma_start(out=m0i, in_=m0.rearrange("(t p) -> p t", p=P))
    m0fc = consts.tile([P, T], f32)
    nc.vector.tensor_copy(out=m0fc, in_=m0i)
    s0 = consts.tile([P, T], f32)
    nc.vector.tensor_scalar(out=s0, in0=m0fc, scalar1=-HUGE, scalar2=HUGE,
                            op0=MUL, op1=mybir.AluOpType.add)

    from concourse.masks import make_identity
    ident_f = consts.tile([P, P], f32)
    make_identity(nc, ident_f)
    ident16 = consts.tile([P, P], f16)
    nc.vector.tensor_copy(out=ident16, in_=ident_f)

    # broadcast wm1row -> wm1 via PE outer product (one-time)
    for c in range(4):
        pw = s_psum.tile([P, 512], f32, tag="sp", name=f"pw{c}")
        nc.tensor.matmul(pw, lhsT=onesrow, rhs=wm1row[:, c * 512:(c + 1) * 512],
                         start=True, stop=True)
        nc.scalar.copy(wm1[:, c * 512:(c + 1) * 512], pw)

    # ---- load q, cast, and transpose into the fp8 DR layout ----
    def prep_pack(src, aug, tr, p4, on_act):
        pt = t_psum.tile([P, 1024], f16, tag="tp")
        for ti in range(4):
            t = p4 * 4 + ti
            st = stage.tile([P, D], f32, tag="st")
            nc.sync.dma_start(
                out=st, in_=src.rearrange("(t p) d -> t p d", p=P)[t]
            )
            st16 = stage16.tile([P, D], f16, tag="st16")
            if on_act:
                nc.scalar.mul(aug[:, t, 0:D], st, INV16)
                nc.scalar.copy(st16, st)
            else:
                nc.vector.tensor_scalar_mul(out=aug[:, t, 0:D], in0=st, scalar1=INV16)
                nc.vector.tensor_copy(out=st16, in_=st)
            for dc in range(2):
                nc.tensor.transpose(
                    pt[:, (ti * 2 + dc) * P:(ti * 2 + dc + 1) * P],
                    st16[:, dc * P:(dc + 1) * P], ident16,
                )
        dst = tr[:, :, p4 * 512:(p4 + 1) * 512]
        dstv = dst.rearrange("p two (t f) -> p t two f", t=4)
        srcv = pt.rearrange("p (t two f) -> p t two f", t=4, two=2)
        nc.vector.tensor_copy(out=dstv, in_=srcv)

    # ---- S matmuls (pure DR, no accumulation) + exp + fused mask-select ----
    def s_stripe(t):
        pss = [s_psum.tile([P, 1024], f32, tag="sp", name=f"sp{_h}") for _h in range(2)]
        for h in range(2):
            for c in range(2):
                off = h * 1024 + c * 512
                nc.tensor.matmul(
                    pss[h][:, c * 512:(c + 1) * 512],
                    lhsT=q0t[:, :, t * P:(t + 1) * P],
                    rhs=q1t[:, :, off:off + 512],
                    start=True, stop=True, perf_mode=DR,
                )
        for h in range(2):
            nc.scalar.activation(
                out=e0[:, t, h * 1024:(h + 1) * 1024], in_=pss[h],
                func=EXP, scale=SCALE2,
            )
        # masked entries -> exactly 0: e0 = min(wm1 + s0[p], e0)
        nc.vector.scalar_tensor_tensor(
            out=e0[:, t, :], in0=wm1, scalar=s0[:, t:t + 1], in1=e0[:, t, :],
            op0=mybir.AluOpType.add, op1=mybir.AluOpType.min,
        )

    # ---- E^T tiles for source stripe i (consumable stripe-by-stripe) ----
    def et_source(i):
        for half in range(2):
            pt = t_psum.tile([P, 1024], f16, tag="tp")
            for si in range(8):
                s = half * 8 + si
                nc.tensor.transpose(
                    pt[:, si * P:(si + 1) * P],
                    e0[:, i, s * P:(s + 1) * P], ident16,
                )
            dst = e1[:, half * 8:(half + 1) * 8, i * P:(i + 1) * P]
            if half == 0:
                nc.vector.tensor_copy(out=dst, in_=pt.rearrange("p (s f) -> p s f", s=8))
            else:
                nc.scalar.copy(dst, pt.rearrange("p (s f) -> p s f", s=8))

    # ---- one pairwise-interleaved pair of out accumulation chains ----
    def out_pair(esrc, raug, odram, j0):
        pos = [o_psum.tile([P, AUGW], f32, tag="op", name=f"op{_k}") for _k in range(2)]
        for t in range(T):
            for k in range(2):
                j = j0 + k
                nc.tensor.matmul(
                    pos[k],
                    lhsT=esrc[:, t, j * P:(j + 1) * P],
                    rhs=raug[:, t, :],
                    start=(t == 0), stop=(t == T - 1),
                )
        for k in range(2):
            j = j0 + k
            rc = small.tile([P, 1], f32, tag="rc")
            nc.vector.reciprocal(rc, pos[k][:, D:D + 1])
            ot = outp.tile([P, D], f32, tag="ot")
            nc.scalar.mul(ot, pos[k][:, 0:D], rc)
            nc.sync.dma_start(out=odram[j * P:(j + 1) * P, :], in_=ot)

    # ---- emission schedule ----
    for p4 in range(4):
        prep_pack(q1, q1a, q1t, p4, on_act=True)
    for p4 in range(4):
        prep_pack(q0, q0a, q0t, p4, on_act=False)
    for t in range(T):
        s_stripe(t)
        et_source(t)
    for j0 in range(0, T, 2):
        out_pair(e0, q0a, out1, j0)
        out_pair(e1, q1a, out0, j0)

_CACHED_NC = None


def _build(debug=False):
    global _CACHED_NC
    if _CACHED_NC is not None:
        return _CACHED_NC
    nc = bacc.Bacc("TRN2", target_bir_lowering=False, debug=False)
    io = {
        "q0": nc.dram_tensor("q0", [L, D], f32, kind="ExternalInput").ap(),
        "q1": nc.dram_tensor("q1", [L, D], f32, kind="ExternalInput").ap(),
        "mask0": nc.dram_tensor("mask0", [L], i32, kind="ExternalInput").ap(),
        "mask1": nc.dram_tensor("mask1", [L], i32, kind="ExternalInput").ap(),
        "out0": nc.dram_tensor("out0", [L, D], f32, kind="ExternalOutput").ap(),
        "out1": nc.dram_tensor("out1", [L, D], f32, kind="ExternalOutput").ap(),
    }
    if debug:
        io["dbg_e0"] = nc.dram_tensor("dbg_e0", [P, 2048], f32, kind="ExternalOutput").ap()
        io["dbg_q0a"] = nc.dram_tensor("dbg_q0a", [P, AUGW], f32, kind="ExternalOutput").ap()
    with tile.TileContext(nc) as tc:
        with ExitStack() as ctx:
            _emit(tc, ctx, io)
    nc.compile()
    _CACHED_NC = nc
    return nc


def run_on_cores(q0, q1, mask0, mask1, trace=False):
    """Run the SPMD kernel; returns (out0, out1, BassKernelResults)."""
    nc = _build()
    in_maps = [
        {
            "q0": np.ascontiguousarray(q0[b], dtype=np.float32),
            "q1": np.ascontiguousarray(q1[b], dtype=np.float32),
            "mask0": np.ascontiguousarray(mask0[b], dtype=np.int32),
            "mask1": np.ascontiguousarray(mask1[b], dtype=np.int32),
        }
        for b in range(B)
    ]
    br = run_bass_kernel_spmd(nc, in_maps, list(range(B)), trace=trace)
    out0 = np.stack([br.results[b]["out0"] for b in range(B)])
    out1 = np.stack([br.results[b]["out1"] for b in range(B)])
    return out0, out1, br


def kernel(q0, q1, len0=None, len1=None, mask0=None, mask1=None, **_):
    q0 = np.asarray(q0, dtype=np.float32)
    q1 = np.asarray(q1, dtype=np.float32)
    mask0 = np.asarray(mask0, dtype=np.int32)
    mask1 = np.asarray(mask1, dtype=np.int32)
    out0, out1, _br = run_on_cores(q0, q1, mask0, mask1, trace=False)
    return out0, out1


# revision 19
# speedup vs baseline: 1.9399x; 1.0338x over previous
"""Trainium2 Bass kernel for nn_Luong_61684320305412 (bidirectional masked
softmax attention, B=8, L0=L1=2048, D=256).

Sharding: data-parallel over batch B across the 8 NeuronCores. Per core:

    S    = q0 @ q1^T + (-448 m0) outer (448 m1)     [fp8e4 operands]
    E    = exp(S/256)                                [fp16, masked entries -> 0]
    out0 = (E  @ [qn1 | 1]) -> cols 0:256 / col 256  (qn = q/16, fp16)
    out1 = (E^T @ [qn0 | 1]) -> cols 0:256 / col 256

Implementation notes:
  - S matmuls run in fp8e4 with MatmulPerfMode.DoubleRow: K=256 in one
    instruction ([128p, 2, .] operand layout), 2 rhs elems/cycle.
  - The mask outer product is a separate K=1 fp8 matmul per 512-chunk
    (lhsT=-128*m0, rhs=+64*m1 -> exp arg -32 -> exp==0 after the fp16 cast;
    more-negative args make the scalar engine's exp produce NaN).
  - E is needed in both orientations (lhsT for both out matmuls); S is
    computed twice (S and S^T) from the same fp8 operands with roles
    swapped - numerically identical to transposing E.
  - exp runs on the scalar engine psum->SBUF in [128,1024] chunks,
    writing E in fp16 (out matmuls are fp16 x fp16 -> f32 psum).
  - Softmax denominators come from two ones-columns appended to the
    fp16 rhs (psum cols 256/257), divided out on the vector engine.
  - Out accumulation chains are emitted pairwise-interleaved across two
    psum banks to hide PSUM read-modify-write latency.
"""

import math
from contextlib import ExitStack

import numpy as np

import concourse.bass as bass
import concourse.tile as tile
from concourse import bacc, mybir
from concourse.bass_utils import run_bass_kernel_spmd

P = 128
B = 8
L = 2048          # L0 == L1
D = 256
T = L // P        # 16 row tiles
AUGW = D + 2      # 258: fp16 q/16 augmented with two ones columns
HUGE = 60000.0    # fp16-exact; mask select: min(HUGE*(1-m1[j]) + HUGE*(1-m0[p]), E)
                  # = 0 iff m0[p]=m1[j]=1 else E (E >= 0 always)
SCALE2 = 1.0 / 256.0   # applied to scores inside exp
INV16 = 1.0 / 16.0

f32 = mybir.dt.float32
f32r = mybir.dt.float32r
f16 = mybir.dt.float16
f8 = mybir.dt.float8e4
i32 = mybir.dt.int32
MUL = mybir.AluOpType.mult
EXP = mybir.ActivationFunctionType.Exp
DR = mybir.MatmulPerfMode.DoubleRow


def _emit(tc: tile.TileContext, ctx: ExitStack, io: dict):
    nc = tc.nc
    q0, q1, m0, m1 = io["q0"], io["q1"], io["mask0"], io["mask1"]
    out0, out1 = io["out0"], io["out1"]

    consts = ctx.enter_context(tc.tile_pool(name="consts", bufs=1))
    stage = ctx.enter_context(tc.tile_pool(name="stage", bufs=4))
    stage16 = ctx.enter_context(tc.tile_pool(name="stage16", bufs=6))
    qpool = ctx.enter_context(tc.tile_pool(name="qpool", bufs=1))
    e_pool = ctx.enter_context(tc.tile_pool(name="e", bufs=1))
    outp = ctx.enter_context(tc.tile_pool(name="outp", bufs=4))
    small = ctx.enter_context(tc.tile_pool(name="small", bufs=4))
    s_psum = ctx.enter_context(tc.tile_pool(name="s_psum", bufs=1, space="PSUM"))
    t_psum = ctx.enter_context(tc.tile_pool(name="t_psum", bufs=2, space="PSUM"))
    o_psum = ctx.enter_context(tc.tile_pool(name="o_psum", bufs=4, space="PSUM"))

    # ---- persistent operand tiles ----
    q0a = qpool.tile([P, T, AUGW], f16)   # q/16 | ones cols (out-matmul rhs)
    q1a = qpool.tile([P, T, AUGW], f16)
    q0t = qpool.tile([P, 2, L], f8)       # raw q, [d%128, d//128, l] DR layout
    q1t = qpool.tile([P, 2, L], f8)
    e0 = e_pool.tile([P, T, L], f16)      # E  [l0, l1]
    e1 = e_pool.tile([P, T, L], f16)      # E^T [l1, l0] (built by PE transpose)

    nc.vector.memset(q0a[:, :, D:AUGW], 1.0)
    nc.vector.memset(q1a[:, :, D:AUGW], 1.0)

    # ---- mask prep ----
    # row tile: wm1[0, j] = HUGE*(1-m1[j]), broadcast to all partitions by a
    # one-time K=1 PE outer product; col tile: s0[p, t] = HUGE*(1-m0[t*128+p])
    m1i = consts.tile([1, L], i32)
    nc.sync.dma_start(out=m1i, in_=m1.rearrange("(o l) -> o l", o=1))
    m1f = consts.tile([1, L], f32)
    nc.vector.tensor_copy(out=m1f, in_=m1i)
    wm1row = consts.tile([1, L], f16)
    nc.vector.tensor_scalar(out=wm1row, in0=m1f, scalar1=-HUGE, scalar2=HUGE,
                            op0=MUL, op1=mybir.AluOpType.add)
    onesrow = consts.tile([1, P], f16)
    nc.vector.memset(onesrow, 1.0)
    wm1 = consts.tile([P, L], f16)

    m0i = consts.tile([P, T], i32)
    nc.sync.dma_start(out=m0i, in_=m0.rearrange("(t p) -> p t", p=P))
    m0fc = consts.tile([P, T], f32)
    nc.vector.tensor_copy(out=m0fc, in_=m0i)
    s0 = consts.tile([P, T], f32)
    nc.vector.tensor_scalar(out=s0, in0=m0fc, scalar1=-HUGE, scalar2=HUGE,
                            op0=MUL, op1=mybir.AluOpType.add)

    from concourse.masks import make_identity
    ident_f = consts.tile([P, P], f32)
    make_identity(nc, ident_f)
    ident16 = consts.tile([P, P], f16)
    nc.vector.tensor_copy(out=ident16, in_=ident_f)

    # broadcast wm1row -> wm1 via PE outer product (one-time)
    for c in range(4):
        pw = s_psum.tile([P, 512], f32, tag="sp", name=f"pw{c}")
        nc.tensor.matmul(pw, lhsT=onesrow, rhs=wm1row[:, c * 512:(c + 1) * 512],
                         start=True, stop=True)
        nc.scalar.copy(wm1[:, c * 512:(c + 1) * 512], pw)

    # ---- load q, cast, and transpose into the fp8 DR layout ----
    def prep_pack(src, aug, tr, p4, on_act):
        pt = t_psum.tile([P, 1024], f16, tag="tp")
        for ti in range(4):
            t = p4 * 4 + ti
            st = stage.tile([P, D], f32, tag="st")
            nc.sync.dma_start(
                out=st, in_=src.rearrange("(t p) d -> t p d", p=P)[t]
            )
            st16 = stage16.tile([P, D], f16, tag="st16")
            if on_act:
                nc.scalar.mul(aug[:, t, 0:D], st, INV16)
                nc.scalar.copy(st16, st)
            else:
                nc.vector.tensor_scalar_mul(out=aug[:, t, 0:D], in0=st, scalar1=INV16)
                nc.vector.tensor_copy(out=st16, in_=st)
            for dc in range(2):
                nc.tensor.transpose(
                    pt[:, (ti * 2 + dc) * P:(ti * 2 + dc + 1) * P],
                    st16[:, dc * P:(dc + 1) * P], ident16,
                )
        dst = tr[:, :, p4 * 512:(p4 + 1) * 512]
        dstv = dst.rearrange("p two (t f) -> p t two f", t=4)
        srcv = pt.rearrange("p (t two f) -> p t two f", t=4, two=2)
        nc.vector.tensor_copy(out=dstv, in_=srcv)

    # ---- S matmuls (pure DR, no accumulation) + exp + fused mask-select ----
    def s_stripe(t):
        for h in range(2):
            ps = s_psum.tile([P, 1024], f32, tag="sp")
            for c in range(2):
                off = h * 1024 + c * 512
                nc.tensor.matmul(
                    ps[:, c * 512:(c + 1) * 512],
                    lhsT=q0t[:, :, t * P:(t + 1) * P],
                    rhs=q1t[:, :, off:off + 512],
                    start=True, stop=True, perf_mode=DR,
                )
            nc.scalar.activation(
                out=e0[:, t, h * 1024:(h + 1) * 1024], in_=ps,
                func=EXP, scale=SCALE2,
            )
        # masked entries -> exactly 0: e0 = min(wm1 + s0[p], e0)
        nc.vector.scalar_tensor_tensor(
            out=e0[:, t, :], in0=wm1, scalar=s0[:, t:t + 1], in1=e0[:, t, :],
            op0=mybir.AluOpType.add, op1=mybir.AluOpType.min,
        )

    # ---- E^T tiles for source stripe i (consumable stripe-by-stripe) ----
    def et_source(i):
        for half in range(2):
            pt = t_psum.tile([P, 1024], f16, tag="tp")
            for si in range(8):
                s = half * 8 + si
                nc.tensor.transpose(
                    pt[:, si * P:(si + 1) * P],
                    e0[:, i, s * P:(s + 1) * P], ident16,
                )
            dst = e1[:, half * 8:(half + 1) * 8, i * P:(i + 1) * P]
            if half == 0:
                nc.vector.tensor_copy(out=dst, in_=pt.rearrange("p (s f) -> p s f", s=8))
            else:
                nc.scalar.copy(dst, pt.rearrange("p (s f) -> p s f", s=8))

    # ---- one pairwise-interleaved pair of out accumulation chains ----
    def out_pair(esrc, raug, odram, j0):
        pos = [o_psum.tile([P, AUGW], f32, tag="op", name=f"op{_k}") for _k in range(2)]
        for t in range(T):
            for k in range(2):
                j = j0 + k
                nc.tensor.matmul(
                    pos[k],
                    lhsT=esrc[:, t, j * P:(j + 1) * P],
                    rhs=raug[:, t, :],
                    start=(t == 0), stop=(t == T - 1),
                )
        for k in range(2):
            j = j0 + k
            rc = small.tile([P, 1], f32, tag="rc")
            nc.vector.reciprocal(rc, pos[k][:, D:D + 1])
            ot = outp.tile([P, D], f32, tag="ot")
            nc.scalar.mul(ot, pos[k][:, 0:D], rc)
            nc.sync.dma_start(out=odram[j * P:(j + 1) * P, :], in_=ot)

    # ---- emission schedule ----
    for p4 in range(4):
        prep_pack(q1, q1a, q1t, p4, on_act=True)
    for p4 in range(4):
        prep_pack(q0, q0a, q0t, p4, on_act=False)
    for t in range(T):
        s_stripe(t)
        et_source(t)
    for j0 in range(0, T, 2):
        out_pair(e0, q0a, out1, j0)
        out_pair(e1, q1a, out0, j0)

_CACHED_NC = None


def _build(debug=False):
    global _CACHED_NC
    if _CACHED_NC is not None:
        return _CACHED_NC
    nc = bacc.Bacc("TRN2", target_bir_lowering=False, debug=False)
    io = {
        "q0": nc.dram_tensor("q0", [L, D], f32, kind="ExternalInput").ap(),
        "q1": nc.dram_tensor("q1", [L, D], f32, kind="ExternalInput").ap(),
        "mask0": nc.dram_tensor("mask0", [L], i32, kind="ExternalInput").ap(),
        "mask1": nc.dram_tensor("mask1", [L], i32, kind="ExternalInput").ap(),
        "out0": nc.dram_tensor("out0", [L, D], f32, kind="ExternalOutput").ap(),
        "out1": nc.dram_tensor("out1", [L, D], f32, kind="ExternalOutput").ap(),
    }
    if debug:
        io["dbg_e0"] = nc.dram_tensor("dbg_e0", [P, 2048], f32, kind="ExternalOutput").ap()
        io["dbg_q0a"] = nc.dram_tensor("dbg_q0a", [P, AUGW], f32, kind="ExternalOutput").ap()
    with tile.TileContext(nc) as tc:
        with ExitStack() as ctx:
            _emit(tc, ctx, io)
    nc.compile()
    _CACHED_NC = nc
    return nc


def run_on_cores(q0, q1, mask0, mask1, trace=False):
    """Run the SPMD kernel; returns (out0, out1, BassKernelResults)."""
    nc = _build()
    in_maps = [
        {
            "q0": np.ascontiguousarray(q0[b], dtype=np.float32),
            "q1": np.ascontiguousarray(q1[b], dtype=np.float32),
            "mask0": np.ascontiguousarray(mask0[b], dtype=np.int32),
            "mask1": np.ascontiguousarray(mask1[b], dtype=np.int32),
        }
        for b in range(B)
    ]
    br = run_bass_kernel_spmd(nc, in_maps, list(range(B)), trace=trace)
    out0 = np.stack([br.results[b]["out0"] for b in range(B)])
    out1 = np.stack([br.results[b]["out1"] for b in range(B)])
    return out0, out1, br


def kernel(q0, q1, len0=None, len1=None, mask0=None, mask1=None, **_):
    q0 = np.asarray(q0, dtype=np.float32)
    q1 = np.asarray(q1, dtype=np.float32)
    mask0 = np.asarray(mask0, dtype=np.int32)
    mask1 = np.asarray(mask1, dtype=np.int32)
    out0, out1, _br = run_on_cores(q0, q1, mask0, mask1, trace=False)
    return out0, out1
